# revision 14
# baseline (speedup 1.0000x reference)
"""GatedLinearAttentionARMA on 8 Trainium2 NeuronCores (Bass/Tile).

B=2, L=512, D=1024, H=16, DH=64.

Both recurrences are linear scans of rank-1 updates, exactly equivalent to
causal quadratic attention:
  O1_t = Gc_t * sum_{s<=t} (Q_t.K_s) * u_s * V_s        (u = silu(K@sw)/Gc)
  E_t  = V_{t+1} - O1_t
  O2_t = sum_{s<t} (q2_{t-1}.k2_s) * E_s
  y    = (O1 + O2) @ cp_w + cp_b

Sharding: core c handles batch b = c//4 and heads (c%4)*4 .. +3.  Each core
emits its heads' d-major partial output yT (1024, 512); the host sums the 4
per-batch partials, transposes, and adds cp_b.

Device program notes (this walrus build allows ONE sync wait per engine
instruction, so the structure is wait-disciplined):
  - ALL inputs ride in one packed (128, TOT) DMA -> single DMA semaphore.
  - ACT (nc.scalar) owns evacuation of projection/output PSUM ("actbig").
  - DVE (nc.vector) owns evacuation of attention PSUM (psD/pso/pot).
  - tiny per-engine anchor ops absorb the DMA wait up front.
  - Q/K/k2 are computed d-major (feature on partitions, time on free axis)
    via swapped matmuls, two heads packed per 128 partitions, so S^T needs
    no transposes and off-diagonal causal blocks need no masking.
"""

import sys

if "/opt/trn_rl_repo" not in sys.path:
    sys.path.insert(0, "/opt/trn_rl_repo")

import numpy as np

B, L, D, H = 2, 512, 1024, 16
DH = D // H            # 64
NCH = L // 128         # 4 time chunks
KC = D // 128          # 8 contraction chunks
N_CORES = 8

# --- packed input column layout (fp32 elements per partition) -------------
_OFF = {}
_c = 0
def _alloc(name, cols):
    global _c
    _OFF[name] = (_c, _c + cols)
    _c += cols
_alloc("xT", KC * L)            # (128, 8, 512)
_alloc("wq", KC * 256)          # (128, 8, 256)
_alloc("wk", KC * 256)
_alloc("wk2", KC * 256)
_alloc("wcp", 4 * D)            # (64p, 4, 1024)  rows 64..127 zero
_alloc("v", 4 * NCH * DH)       # (128, 4, 4, 64)
_alloc("vs", 4 * NCH * DH)
_alloc("gcb", 4 * L)            # (64p, 4, 512)   rows 64..127 zero
_alloc("u", 4 * NCH)            # (128, 4, 4)
_alloc("gc", 4 * NCH)
_alloc("bq", 2)
_alloc("bk", 2)
_alloc("bqr", 2)                # 0.98*bq
_alloc("bk2s", 2)               # c2*k2_b
_alloc("mi", 128)               # incl causal mask (p<=j)
_alloc("ms8", 128)              # strict causal mask * 0.125
TOT = _c

_CACHE = {}


def _build_nc():
    import concourse.bass as bass
    import concourse.mybir as mybir
    from concourse.tile import TileContext, add_dep_helper

    f32 = mybir.dt.float32
    nc = bass.Bass()

    inp = nc.dram_tensor("inp", [128, TOT], f32, kind="ExternalInput")
    yT = nc.dram_tensor("yT", [D, L], f32, kind="ExternalOutput")

    C2 = 0.02 / float(np.sqrt(D))
    Ident = mybir.ActivationFunctionType.Identity
    Sig = mybir.ActivationFunctionType.Sigmoid
    Rel = mybir.ActivationFunctionType.Relu
    Cpy = mybir.ActivationFunctionType.Copy
    MUL = mybir.AluOpType.mult

    with TileContext(nc) as tc:
        with (
            tc.tile_pool(name="cst", bufs=1) as cst,
            tc.tile_pool(name="qk", bufs=1) as qk,
            tc.tile_pool(name="att", bufs=8) as att,
            tc.tile_pool(name="sm", bufs=4) as sm,
            tc.tile_pool(name="psA", bufs=2, space="PSUM") as psA,
            tc.tile_pool(name="psD", bufs=3, space="PSUM") as psD,
            tc.tile_pool(name="pot", bufs=3, space="PSUM") as pot,
        ):
            mega = cst.tile([128, TOT], f32)
            nc.gpsimd.dma_start(out=mega[:], in_=inp[:])

            def view(name, *shape):
                a, b = _OFF[name]
                ap = mega[:, a:b]
                if len(shape) > 1:
                    ap = ap.rearrange(
                        "p (" + " ".join(f"d{i}" for i in range(len(shape))) + ") -> p "
                        + " ".join(f"d{i}" for i in range(len(shape))),
                        **{f"d{i}": s for i, s in enumerate(shape)},
                    )
                return ap

            v_xT = view("xT", KC, L)
            v_wq = view("wq", KC, 256)
            v_wk = view("wk", KC, 256)
            v_wk2 = view("wk2", KC, 256)
            v_wcp = view("wcp", 4, D)
            v_v = view("v", 4, NCH, DH)
            v_vs = view("vs", 4, NCH, DH)
            v_gcb = view("gcb", 4, L)
            v_u = view("u", 4, NCH)
            v_gc = view("gc", 4, NCH)
            v_bq = view("bq", 2)
            v_bk = view("bk", 2)
            v_bqr = view("bqr", 2)
            v_bk2s = view("bk2s", 2)
            v_mi = view("mi", 128)
            v_ms8 = view("ms8", 128)

            # per-engine anchors: absorb the one DMA wait up front
            anc = sm.tile([1, 2], f32, tag="anc", bufs=1)
            nc.scalar.copy(anc[0:1, 0:1], mega[0:1, 0:1])
            nc.vector.tensor_copy(anc[0:1, 1:2], mega[0:1, 0:1])

            # ---- projections: d-major QT/KT/k2T (2 heads per tile) ------
            QT = qk.tile([128, 2, L], f32)
            KT = qk.tile([128, 2, L], f32)
            K2T = qk.tile([128, 2, L], f32)
            RP = qk.tile([128, 2, L], f32)
            Q2M = qk.tile([128, 2, L], f32)   # min(Q, 0.02Q)

            for hp in range(2):
                mcol = slice(hp * 128, (hp + 1) * 128)
                pk2 = psA.tile([128, L], f32, tag="pbig")
                for kc in range(KC):
                    nc.tensor.matmul(pk2[:], v_wk2[:, kc, mcol], v_xT[:, kc, :],
                                     start=(kc == 0), stop=(kc == KC - 1))
                nc.scalar.activation(K2T[:, hp, :], pk2[:], Sig,
                                     bias=v_bk2s[:, hp:hp + 1], scale=C2)

                pq = psA.tile([128, L], f32, tag="pbig")
                for kc in range(KC):
                    nc.tensor.matmul(pq[:], v_wq[:, kc, mcol], v_xT[:, kc, :],
                                     start=(kc == 0), stop=(kc == KC - 1))
                nc.scalar.activation(RP[:, hp, :], pq[:], Rel,
                                     bias=v_bqr[:, hp:hp + 1], scale=0.98)
                nc.scalar.activation(QT[:, hp, :], pq[:], Ident,
                                     bias=v_bq[:, hp:hp + 1], scale=1.0)

                pk = psA.tile([128, L], f32, tag="pbig")
                for kc in range(KC):
                    nc.tensor.matmul(pk[:], v_wk[:, kc, mcol], v_xT[:, kc, :],
                                     start=(kc == 0), stop=(kc == KC - 1))
                nc.scalar.activation(KT[:, hp, :], pk[:], Ident,
                                     bias=v_bk[:, hp:hp + 1], scale=1.0)

                # q2m = Q - 0.98*relu(Q) = min(Q, 0.02*Q)   (true q2 * 8)
                nc.vector.tensor_sub(Q2M[:, hp, :], QT[:, hp, :], RP[:, hp, :])

            # ---- per-head attention ------------------------------------
            outs = []
            for h in range(4):
                hp, r0 = h // 2, (h % 2) * 64
                rows = slice(r0, r0 + 64)
                qt = QT[rows, hp, :]
                kt = KT[rows, hp, :]
                k2t = K2T[rows, hp, :]
                q2m = Q2M[rows, hp, :]

                # branch 1: S^T chunks -> A1 (u-scaled, causal incl)
                # cs=2 and cs=3 share one PSUM bank (column-packed) so the
                # three banks cover the whole phase with no bank recycling
                # inside it (walrus allows one sync wait per instruction).
                pa = psD.tile([128, L], f32, tag="pd", name=f"pa1_{h}")
                pb = psD.tile([128, L], f32, tag="pd", name=f"pb1_{h}")
                pc = psD.tile([128, L], f32, tag="pd", name=f"pc1_{h}")
                s_plan = [(0, pa, 0), (1, pb, 0), (2, pc, 0), (3, pc, 256)]
                for cs, ps1, cb in s_plan:
                    c0 = cs * 128
                    nc.tensor.matmul(ps1[:, cb:cb + L - c0], kt[:, c0:c0 + 128],
                                     qt[:, c0:L], start=(cb == 0), stop=True,
                                     skip_group_check=True)
                A1 = []
                for cs, ps1, cb in s_plan:
                    c0 = cs * 128
                    w = L - c0
                    a1 = att.tile([128, L], f32, tag="a1", name=f"a1_{h}_{cs}")
                    usl = v_u[:, h, cs:cs + 1]
                    nc.vector.tensor_tensor(a1[:, c0:c0 + 128],
                                            ps1[:, cb:cb + 128], v_mi[:], MUL)
                    nc.vector.tensor_scalar(a1[:, c0:c0 + 128], a1[:, c0:c0 + 128],
                                            usl, None, MUL)
                    if cs < NCH - 1:
                        nc.vector.tensor_scalar(a1[:, c0 + 128:L],
                                                ps1[:, cb + 128:cb + w],
                                                usl, None, MUL)
                    A1.append(a1)

                # PE wait-absorber: a 1x1 bf16 chip written by DVE after the
                # A1 copies; ldweights touches no PSUM bank, so it can carry
                # the single DVE wait and the O1/O1T matmuls below need only
                # their PSUM-bank self-waits.
                wtb = sm.tile([1, 1], mybir.dt.bfloat16, tag="wtb", bufs=8,
                              name=f"wtb_{h}")
                nc.vector.tensor_copy(wtb[:], A1[3][0:1, 508:509])
                ldw = nc.tensor.ldweights(wtb[:])

                # O1 (t-major) + E
                Et = att.tile([128, NCH, DH], f32, tag="et", bufs=2,
                              name=f"et_{h}")
                for ct in range(NCH):
                    t0 = ct * 128
                    if ct > 0:
                        wtc = sm.tile([1, 1], mybir.dt.bfloat16, tag="wtb",
                                      bufs=8, name=f"wtc_{h}_{ct}")
                        nc.vector.tensor_copy(wtc[:], Et[0:1, ct - 1, 0:1])
                        ldw = nc.tensor.ldweights(wtc[:])
                    po1 = psD.tile([128, DH], f32, tag="pd", name=f"po1_{h}_{ct}")
                    for cs in range(ct + 1):
                        mm = nc.tensor.matmul(po1[:], A1[cs][:, t0:t0 + 128],
                                              v_v[:, h, cs, :],
                                              start=(cs == 0), stop=(cs == ct))
                        if cs == 0:
                            add_dep_helper(mm.ins, ldw.ins, sync=False,
                                           reason="absorber before po1 group")
                    o1 = sm.tile([128, DH], f32, tag="o1", bufs=4,
                                 name=f"o1_{h}_{ct}")
                    nc.vector.tensor_scalar(o1[:], po1[:],
                                            v_gc[:, h, ct:ct + 1], None, MUL)
                    nc.vector.tensor_sub(Et[:, ct, :], v_vs[:, h, ct, :], o1[:])

                # O1^T (d-major, not yet gc-scaled)
                po1T = pot.tile([64, L], f32, tag="pot", name=f"po1T_{h}")
                for cs in range(NCH):
                    c0 = cs * 128
                    nc.tensor.matmul(po1T[:, c0:L], v_v[:, h, cs, :],
                                     A1[cs][:, c0:L],
                                     start=(cs == 0), stop=(cs == NCH - 1))

                # branch 2: S2^T = k2 (x) q2(shifted), strict mask, *1/8
                pa2 = psD.tile([128, L], f32, tag="pd", name=f"pa2_{h}")
                pb2 = psD.tile([128, L], f32, tag="pd", name=f"pb2_{h}")
                pc2 = psD.tile([128, L], f32, tag="pd", name=f"pc2_{h}")
                s2_plan = [(0, pa2, 0), (1, pb2, 0), (2, pc2, 0), (3, pc2, 256)]
                for cs, ps2, cb in s2_plan:
                    c0 = cs * 128
                    t0 = max(1, c0)
                    nc.tensor.matmul(ps2[:, cb:cb + L - t0], k2t[:, c0:c0 + 128],
                                     q2m[:, t0 - 1:L - 1], start=(cb == 0),
                                     stop=True, skip_group_check=True)
                A2 = []
                for cs, ps2, cb in s2_plan:
                    c0 = cs * 128
                    t0 = max(1, c0)
                    w = L - t0
                    a2 = att.tile([128, L], f32, tag="a2", name=f"a2_{h}_{cs}")
                    if cs == 0:
                        # tile-local col j corresponds to t = j + 1
                        nc.vector.tensor_tensor(a2[:, 1:128], ps2[:, 0:127],
                                                v_ms8[:, 1:128], MUL)
                        nc.vector.memset(a2[:, 0:1], 0.0)
                        nc.vector.tensor_scalar(a2[:, 128:L], ps2[:, 127:w],
                                                0.125, None, MUL)
                    else:
                        nc.vector.tensor_tensor(a2[:, c0:c0 + 128],
                                                ps2[:, cb:cb + 128], v_ms8[:], MUL)
                        if cs < NCH - 1:
                            nc.vector.tensor_scalar(a2[:, c0 + 128:L],
                                                    ps2[:, cb + 128:cb + w],
                                                    0.125, None, MUL)
                    A2.append(a2)

                # O2^T (d-major)
                po2T = pot.tile([64, L], f32, tag="pot", name=f"po2T_{h}")
                for cs in range(NCH):
                    c0 = cs * 128
                    nc.tensor.matmul(po2T[:, c0:L], Et[:, cs, :], A2[cs][:, c0:L],
                                     start=(cs == 0), stop=(cs == NCH - 1))

                # combine: outT_h = gc_t * O1^T + O2^T
                m1 = sm.tile([64, L], f32, tag="m1", bufs=2, name=f"m1_{h}")
                nc.vector.tensor_tensor(m1[:], po1T[:], v_gcb[0:64, h, :], MUL)
                ot = att.tile([64, L], f32, tag="ot", bufs=4, name=f"ot_{h}")
                nc.vector.tensor_tensor(ot[:], po2T[:], m1[:],
                                        mybir.AluOpType.add)
                outs.append(ot)

            # ---- output projection yT[n, t] = sum_h wcp[j(h), n] outT_h[j, t]
            # ACT mirror of head 0's output: the first matmul of every py
            # group then waits only on ACT (merged with the psA bank WAR).
            ot0a = qk.tile([64, L], f32)
            nc.scalar.copy(ot0a[:], outs[0][:])
            rhs_h = [ot0a] + outs[1:]
            ysb = qk.tile([128, KC, L], f32)
            for nci in range(KC):
                n0 = nci * 128
                py = psA.tile([128, L], f32, tag="pbig", name=f"py_{nci}")
                for h in range(4):
                    nc.tensor.matmul(py[:], v_wcp[0:64, h, n0:n0 + 128],
                                     rhs_h[h][:], start=(h == 0), stop=(h == 3))
                nc.scalar.copy(ysb[:, nci, :], py[:])
            nc.gpsimd.dma_start(
                out=yT.ap().rearrange("(c p) t -> p c t", p=128), in_=ysb[:])

    # this walrus build allows ONE sync wait per instruction; Tile's final
    # drain carries the whole vector clock, so split it into a chain of
    # single-wait drains (the SP sequencer executes them in order).
    nsplit = 0
    for bb in nc.m.functions[0].blocks:
        new_insts = []
        for inst in bb.instructions:
            si = getattr(inst, "sync_info", None)
            if si is not None and si.on_wait and len(si.on_wait) > 1:
                waits = list(si.on_wait)
                for j, w in enumerate(waits[:-1]):
                    d = mybir.InstDrain(
                        name=f"{inst.name}_sw{j}", engine=inst.engine,
                        ins=[], outs=[],
                        sync_info=mybir.SyncInfo(on_wait=[w], on_update=[]))
                    new_insts.append(d)
                    nsplit += 1
                si.on_wait = [waits[-1]]
            new_insts.append(inst)
        bb.instructions = new_insts
    return nc


# ----------------------------------------------------------------- host side
def _sigmoid(z):
    return 1.0 / (1.0 + np.exp(-z))


def _pack_core(core, x, Gc, U, wq_h, wk_h, wk2_h, wcp_h, bq_h, bk_h, bk2s_h,
               mi, ms8):
    b = core // 4
    h0 = (core % 4) * 4

    pk = np.zeros((128, TOT), dtype=np.float32)

    def put(name, arr):
        a, bb = _OFF[name]
        pk[:arr.shape[0], a:bb] = arr.reshape(arr.shape[0], -1)

    xb = x[b]                                        # (L, D)
    put("xT", xb.T.reshape(KC, 128, L).transpose(1, 0, 2))
    put("wq", wq_h[core])
    put("wk", wk_h[core])
    put("wk2", wk2_h[core])
    put("wcp", wcp_h[core])

    hsl = slice(h0 * DH, (h0 + 4) * DH)
    vh = xb[:, hsl].reshape(L, 4, DH)
    put("v", vh.reshape(NCH, 128, 4, DH).transpose(1, 2, 0, 3))
    vsh = np.zeros_like(vh)
    vsh[:-1] = vh[1:]
    put("vs", vsh.reshape(NCH, 128, 4, DH).transpose(1, 2, 0, 3))

    gch = Gc[b, :, h0:h0 + 4]                        # (L, 4)
    uh = U[b, :, h0:h0 + 4]
    put("gcb", np.broadcast_to(gch.T[None, :, :], (64, 4, L)))
    put("u", uh.reshape(NCH, 128, 4).transpose(1, 2, 0))
    put("gc", gch.reshape(NCH, 128, 4).transpose(1, 2, 0))
    put("bq", bq_h[core])
    put("bk", bk_h[core])
    put("bqr", 0.98 * bq_h[core])
    put("bk2s", bk2s_h[core])
    put("mi", mi)
    put("ms8", ms8)
    return pk


def kernel(**inputs):
    inputs = {k: np.asarray(v, dtype=np.float32) for k, v in inputs.items()}
    x = inputs["x"]
    q1_w, q1_b = inputs["q1_w"], inputs["q1_b"]
    k1_w, k1_b = inputs["k1_w"], inputs["k1_b"]
    k2_w, k2_b = inputs["k2_w"], inputs["k2_b"]
    gw_w, gw_b = inputs["gw_w"], inputs["gw_b"]
    sw_w, sw_b = inputs["sw_w"], inputs["sw_b"]
    cp_w, cp_b = inputs["cp_w"], inputs["cp_b"]

    # host scalars: gate cumprod Gc and u = silu(K@sw)/Gc  (tiny matvecs)
    xh = x.reshape(B, L, H, DH)
    zg = np.einsum("blhd,d->blh", xh, gw_w[:, 0], optimize=True) + gw_b[0]
    G = _sigmoid(zg)
    log_cp = np.clip(np.cumsum(np.log(np.clip(G, 1e-6, None)), axis=1),
                     -30.0, 30.0)
    Gc = (np.exp(log_cp) + 1e-6).astype(np.float32)        # (B, L, H)

    Wr = (k1_w.reshape(D, H, DH) @ sw_w[:, 0]).astype(np.float32)
    Cr = (k1_b.reshape(H, DH) @ sw_w[:, 0]) + sw_b[0]
    zr = x @ Wr + Cr
    R = zr * _sigmoid(zr)
    U = (R / Gc).astype(np.float32)

    idx = np.arange(128)
    mi = (idx[:, None] <= idx[None, :]).astype(np.float32)
    ms8 = 0.125 * (idx[:, None] < idx[None, :]).astype(np.float32)
    c2 = np.float32(0.02 / np.sqrt(np.float32(D)))

    wq_h, wk_h, wk2_h, wcp_h, bq_h, bk_h, bk2s_h = {}, {}, {}, {}, {}, {}, {}
    for core in range(N_CORES):
        h0 = (core % 4) * 4
        hsl = slice(h0 * DH, (h0 + 4) * DH)
        wq_h[core] = q1_w[:, hsl].reshape(KC, 128, 256).transpose(1, 0, 2)
        wk_h[core] = k1_w[:, hsl].reshape(KC, 128, 256).transpose(1, 0, 2)
        wk2_h[core] = k2_w[:, hsl].reshape(KC, 128, 256).transpose(1, 0, 2)
        wcp_h[core] = cp_w[hsl, :].reshape(4, 64, D).transpose(1, 0, 2)
        bq_h[core] = q1_b[hsl].reshape(2, 128).T
        bk_h[core] = k1_b[hsl].reshape(2, 128).T
        bk2s_h[core] = (k2_b[hsl] * c2).reshape(2, 128).T

    in_maps = [
        {"inp": _pack_core(core, x, Gc, U, wq_h, wk_h, wk2_h, wcp_h,
                           bq_h, bk_h, bk2s_h, mi, ms8)}
        for core in range(N_CORES)
    ]

    res = _run(in_maps)

    y = np.empty((B, L, D), dtype=np.float32)
    for b in range(B):
        acc = res[4 * b]["yT"].astype(np.float32)
        for c in range(4 * b + 1, 4 * b + 4):
            acc = acc + res[c]["yT"]
        y[b] = acc.T
    y += cp_b
    return y


def _run(in_maps, trace=False):
    if "nc" not in _CACHE:
        _CACHE["nc"] = _build_nc()
    from concourse.bass_utils import run_bass_kernel_spmd
    r = run_bass_kernel_spmd(_CACHE["nc"], in_maps,
                             core_ids=list(range(N_CORES)), trace=trace)
    _CACHE["last"] = r
    return r.results


# revision 21
# speedup vs baseline: 1.0216x; 1.0216x over previous
"""GatedLinearAttentionARMA on 8 Trainium2 NeuronCores (Bass/Tile).

B=2, L=512, D=1024, H=16, DH=64.

Both recurrences are linear scans of rank-1 updates, exactly equivalent to
causal quadratic attention:
  O1_t = Gc_t * sum_{s<=t} (Q_t.K_s) * u_s * V_s        (u = silu(K@sw)/Gc)
  E_t  = V_{t+1} - O1_t
  O2_t = sum_{s<t} (q2_{t-1}.k2_s) * E_s
  y    = (O1 + O2) @ cp_w + cp_b

Sharding: core c handles batch b = c//4 and heads (c%4)*4 .. +3.  Each core
emits its heads' d-major partial output yT (1024, 512); the host sums the 4
per-batch partials, transposes, and adds cp_b.

Device program notes (this walrus build allows ONE sync wait per engine
instruction, so the structure is wait-disciplined):
  - ALL inputs ride in one packed (128, TOT) DMA -> single DMA semaphore.
  - ACT (nc.scalar) owns evacuation of projection/output PSUM ("actbig").
  - DVE (nc.vector) owns evacuation of attention PSUM (psD/pso/pot).
  - tiny per-engine anchor ops absorb the DMA wait up front.
  - Q/K/k2 are computed d-major (feature on partitions, time on free axis)
    via swapped matmuls, two heads packed per 128 partitions, so S^T needs
    no transposes and off-diagonal causal blocks need no masking.
"""

import sys

if "/opt/trn_rl_repo" not in sys.path:
    sys.path.insert(0, "/opt/trn_rl_repo")

import numpy as np

B, L, D, H = 2, 512, 1024, 16
DH = D // H            # 64
NCH = L // 128         # 4 time chunks
KC = D // 128          # 8 contraction chunks
N_CORES = 8

# --- packed input column layout (fp32 elements per partition) -------------
_OFF = {}
_c = 0
def _alloc(name, cols):
    global _c
    _OFF[name] = (_c, _c + cols)
    _c += cols
_alloc("xT", KC * L)            # (128, 8, 512)
_alloc("wq", KC * 256)          # (128, 8, 256)
_alloc("u", 4 * NCH)            # (128, 4, 4)
_alloc("gc", 4 * NCH)
_alloc("bq", 2)
_alloc("bk", 2)
_alloc("bqr", 2)                # 0.98*bq
_alloc("bk2s", 2)               # c2*k2_b
_alloc("mi", 128)               # incl causal mask (p<=j)
_alloc("ms8", 128)              # strict causal mask * 0.125
R0_END = _c                     # dma range 0: xT+wq+scalars/masks
_alloc("wk", KC * 256)
_alloc("wk2", KC * 256)
R1_END = _c                     # dma range 1: wk+wk2
_alloc("v", 4 * NCH * DH)       # (128, 4, 4, 64)
_alloc("vs", 4 * NCH * DH)
_alloc("gcb", 4 * L)            # (64p, 4, 512)   rows 64..127 zero
R2_END = _c                     # dma range 2: attention operands
_alloc("wcp", 4 * D)            # (64p, 4, 1024)  rows 64..127 zero
TOT = _c                        # dma range 3: wcp

_CACHE = {}


def _build_nc(split_waits=True):
    import concourse.bass as bass
    import concourse.mybir as mybir
    from concourse.tile import TileContext, add_dep_helper

    f32 = mybir.dt.float32
    f32r = mybir.dt.float32r
    nc = bass.Bass()

    inp = nc.dram_tensor("inp", [128, TOT], f32r, kind="ExternalInput")
    yT = nc.dram_tensor("yT", [D, L], f32, kind="ExternalOutput")

    C2 = 0.02 / float(np.sqrt(D))
    Ident = mybir.ActivationFunctionType.Identity
    Sig = mybir.ActivationFunctionType.Sigmoid
    Rel = mybir.ActivationFunctionType.Relu
    Cpy = mybir.ActivationFunctionType.Copy
    MUL = mybir.AluOpType.mult

    with TileContext(nc) as tc:
        with (
            tc.tile_pool(name="cst", bufs=1) as cst,
            tc.tile_pool(name="qk", bufs=1) as qk,
            tc.tile_pool(name="att", bufs=8) as att,
            tc.tile_pool(name="sm", bufs=4) as sm,
            tc.tile_pool(name="psA", bufs=2, space="PSUM") as psA,
            tc.tile_pool(name="psD", bufs=3, space="PSUM") as psD,
            tc.tile_pool(name="pot", bufs=3, space="PSUM") as pot,
        ):
            mega = cst.tile([128, TOT], f32r)
            nc.sync.dma_start(out=mega[:, 0:R0_END], in_=inp[:, 0:R0_END])
            nc.scalar.dma_start(out=mega[:, R0_END:R1_END],
                                in_=inp[:, R0_END:R1_END])
            nc.gpsimd.dma_start(out=mega[:, R1_END:R2_END],
                                in_=inp[:, R1_END:R2_END])
            nc.gpsimd.dma_start(out=mega[:, R2_END:TOT],
                                in_=inp[:, R2_END:TOT])

            def view(name, *shape):
                a, b = _OFF[name]
                ap = mega[:, a:b]
                if len(shape) > 1:
                    ap = ap.rearrange(
                        "p (" + " ".join(f"d{i}" for i in range(len(shape))) + ") -> p "
                        + " ".join(f"d{i}" for i in range(len(shape))),
                        **{f"d{i}": s for i, s in enumerate(shape)},
                    )
                return ap

            v_xT = view("xT", KC, L)
            v_wq = view("wq", KC, 256)
            v_wk = view("wk", KC, 256)
            v_wk2 = view("wk2", KC, 256)
            v_wcp = view("wcp", 4, D)
            v_v = view("v", 4, NCH, DH)
            v_vs = view("vs", 4, NCH, DH)
            v_gcb = view("gcb", 4, L)
            v_u = view("u", 4, NCH)
            v_gc = view("gc", 4, NCH)
            v_bq = view("bq", 2)
            v_bk = view("bk", 2)
            v_bqr = view("bqr", 2)
            v_bk2s = view("bk2s", 2)
            v_mi = view("mi", 128)
            v_ms8 = view("ms8", 128)

            # per-engine anchors: absorb one DMA wait each up front
            bf16 = mybir.dt.bfloat16
            anc = sm.tile([1, 2], f32, tag="anc", bufs=1)
            nc.scalar.copy(anc[0:1, 0:1], v_bq[0:1, 0:1])
            nc.vector.tensor_copy(anc[0:1, 1:2], v_mi[0:1, 0:1])
            ldw_r2 = nc.tensor.ldweights(v_v[0:1, 0, 0, :].bitcast(bf16)[:, 0:1])
            ldw_r3 = nc.tensor.ldweights(v_wcp[0:1, 0, :].bitcast(bf16)[:, 0:1])

            # f32 copies of the per-partition scalar block (tensor_scalar and
            # activation bias operands must be plain float32)
            sc = cst.tile([128, 32], f32)    # u(16) + gc(16)
            nc.vector.tensor_copy(sc[:], mega[:, _OFF["u"][0]:_OFF["gc"][1]])
            scA = cst.tile([128, 8], f32)    # bq bk bqr bk2s
            nc.scalar.copy(scA[:], mega[:, _OFF["bq"][0]:_OFF["bk2s"][1]])
            v_u = sc[:, 0:16].rearrange("p (h c) -> p h c", h=4)
            v_gc = sc[:, 16:32].rearrange("p (h c) -> p h c", h=4)
            v_bq = scA[:, 0:2]
            v_bk = scA[:, 2:4]
            v_bqr = scA[:, 4:6]
            v_bk2s = scA[:, 6:8]

            def MM(out, lhsT, rhs, **kw):
                return nc.tensor.matmul(out, lhsT, rhs, **kw)

            # ---- projections: d-major QT/KT/k2T (2 heads per tile) ------
            QT = qk.tile([128, 2, L], f32r)
            KT = qk.tile([128, 2, L], f32r)
            K2T = qk.tile([128, 2, L], f32r)
            RP = qk.tile([128, 2, L], f32r)
            Q2M = qk.tile([128, 2, L], f32r)   # min(Q, 0.02Q)
            Q2S = qk.tile([128, 2, L], f32r)   # Q2M shifted right by one

            for hp in range(2):
                mcol = slice(hp * 128, (hp + 1) * 128)
                pk2 = psA.tile([128, L], f32, tag="pbig")
                for kc in range(KC):
                    MM(pk2[:], v_wk2[:, kc, mcol], v_xT[:, kc, :],
                                     start=(kc == 0), stop=(kc == KC - 1))
                nc.scalar.activation(K2T[:, hp, :], pk2[:], Sig,
                                     bias=v_bk2s[:, hp:hp + 1], scale=C2)

                pq = psA.tile([128, L], f32, tag="pbig")
                for kc in range(KC):
                    MM(pq[:], v_wq[:, kc, mcol], v_xT[:, kc, :],
                                     start=(kc == 0), stop=(kc == KC - 1))
                nc.scalar.activation(RP[:, hp, :], pq[:], Rel,
                                     bias=v_bqr[:, hp:hp + 1], scale=0.98)
                nc.scalar.activation(QT[:, hp, :], pq[:], Ident,
                                     bias=v_bq[:, hp:hp + 1], scale=1.0)

                pk = psA.tile([128, L], f32, tag="pbig")
                for kc in range(KC):
                    MM(pk[:], v_wk[:, kc, mcol], v_xT[:, kc, :],
                                     start=(kc == 0), stop=(kc == KC - 1))
                nc.scalar.activation(KT[:, hp, :], pk[:], Ident,
                                     bias=v_bk[:, hp:hp + 1], scale=1.0)

                # q2m = Q - 0.98*relu(Q) = min(Q, 0.02*Q)   (true q2 * 8)
                nc.vector.tensor_sub(Q2M[:, hp, :], QT[:, hp, :], RP[:, hp, :])
                nc.vector.tensor_copy(Q2S[:, hp, 1:L], Q2M[:, hp, 0:L - 1])
                nc.vector.tensor_scalar(Q2S[:, hp, 0:1], QT[:, hp, 0:1],
                                        0.0, None, MUL)

            # ---- per-head attention ------------------------------------
            outs = []
            for h in range(4):
                hp, r0 = h // 2, (h % 2) * 64
                rows = slice(r0, r0 + 64)
                qt = QT[rows, hp, :]
                kt = KT[rows, hp, :]
                k2t = K2T[rows, hp, :]
                q2s = Q2S[rows, hp, :]

                # branch 1: S^T chunks -> A1 (u-scaled, causal incl)
                # cs=2 and cs=3 share one PSUM bank (column-packed) so the
                # three banks cover the whole phase with no bank recycling
                # inside it (walrus allows one sync wait per instruction).
                pa = psD.tile([128, L], f32, tag="pd", name=f"pa1_{h}")
                pb = psD.tile([128, L], f32, tag="pd", name=f"pb1_{h}")
                pc = psD.tile([128, L], f32, tag="pd", name=f"pc1_{h}")
                s_plan = [(0, pa, 0), (1, pb, 0), (2, pc, 0), (3, pc, 256)]
                for cs, ps1, cb in s_plan:
                    c0 = cs * 128
                    MM(ps1[:, cb:cb + L - c0], kt[:, c0:c0 + 128],
                                     qt[:, c0:L], start=(cb == 0), stop=True,
                                     skip_group_check=True)
                A1 = []
                for cs, ps1, cb in s_plan:
                    c0 = cs * 128
                    w = L - c0
                    a1 = att.tile([128, L], f32r, tag="a1", name=f"a1_{h}_{cs}")
                    usl = v_u[:, h, cs:cs + 1]
                    nc.vector.tensor_tensor(a1[:, c0:c0 + 128],
                                            ps1[:, cb:cb + 128], v_mi[:], MUL)
                    nc.vector.tensor_scalar(a1[:, c0:c0 + 128], a1[:, c0:c0 + 128],
                                            usl, None, MUL)
                    if cs < NCH - 1:
                        nc.vector.tensor_scalar(a1[:, c0 + 128:L],
                                                ps1[:, cb + 128:cb + w],
                                                usl, None, MUL)
                    A1.append(a1)

                # PE wait-absorber: a 1x1 bf16 chip written by DVE after the
                # A1 copies; ldweights touches no PSUM bank, so it can carry
                # the single DVE wait and the O1/O1T matmuls below need only
                # their PSUM-bank self-waits.
                wtb = sm.tile([1, 1], mybir.dt.bfloat16, tag="wtb", bufs=8,
                              name=f"wtb_{h}")
                nc.vector.tensor_copy(wtb[:], A1[3][0:1, 508:509])
                ldw = nc.tensor.ldweights(wtb[:])

                # O1 (t-major) + E
                Et = att.tile([128, NCH, DH], f32r, tag="et", bufs=2,
                              name=f"et_{h}")
                for ct in range(NCH):
                    t0 = ct * 128
                    if ct > 0:
                        wtc = sm.tile([1, 1], mybir.dt.bfloat16, tag="wtb",
                                      bufs=8, name=f"wtc_{h}_{ct}")
                        nc.vector.tensor_copy(wtc[:], Et[0:1, ct - 1, 0:1])
                        ldw = nc.tensor.ldweights(wtc[:])
                    po1 = psD.tile([128, DH], f32, tag="pd", name=f"po1_{h}_{ct}")
                    for cs in range(ct + 1):
                        mm = MM(po1[:], A1[cs][:, t0:t0 + 128],
                                              v_v[:, h, cs, :],
                                              start=(cs == 0), stop=(cs == ct))
                        if cs == 0:
                            add_dep_helper(mm.ins, ldw.ins, sync=False,
                                           reason="absorber before po1 group")
                        if cs == 0 and h == 0 and ct == 0:
                            add_dep_helper(mm.ins, ldw_r2.ins, sync=False,
                                           reason="r2 anchor before attention")
                    o1 = sm.tile([128, DH], f32, tag="o1", bufs=4,
                                 name=f"o1_{h}_{ct}")
                    nc.vector.tensor_scalar(o1[:], po1[:],
                                            v_gc[:, h, ct:ct + 1], None, MUL)
                    nc.vector.tensor_sub(Et[:, ct, :], v_vs[:, h, ct, :], o1[:])

                # O1^T (d-major, not yet gc-scaled)
                po1T = pot.tile([64, L], f32, tag="pot", name=f"po1T_{h}")
                for cs in range(NCH):
                    c0 = cs * 128
                    MM(po1T[:, c0:L], v_v[:, h, cs, :],
                                     A1[cs][:, c0:L],
                                     start=(cs == 0), stop=(cs == NCH - 1))

                # branch 2: S2^T = k2 (x) q2(shifted), strict mask, *1/8
                pa2 = psD.tile([128, L], f32, tag="pd", name=f"pa2_{h}")
                pb2 = psD.tile([128, L], f32, tag="pd", name=f"pb2_{h}")
                pc2 = psD.tile([128, L], f32, tag="pd", name=f"pc2_{h}")
                s2_plan = [(0, pa2, 0), (1, pb2, 0), (2, pc2, 0), (3, pc2, 256)]
                for cs, ps2, cb in s2_plan:
                    c0 = cs * 128
                    MM(ps2[:, cb:cb + L - c0], k2t[:, c0:c0 + 128],
                       q2s[:, c0:L], start=(cb == 0), stop=True,
                       skip_group_check=True)
                A2 = []
                for cs, ps2, cb in s2_plan:
                    c0 = cs * 128
                    w = L - c0
                    a2 = att.tile([128, L], f32r, tag="a2", name=f"a2_{h}_{cs}")
                    nc.vector.tensor_tensor(a2[:, c0:c0 + 128],
                                            ps2[:, cb:cb + 128], v_ms8[:], MUL)
                    if cs < NCH - 1:
                        nc.vector.tensor_scalar(a2[:, c0 + 128:L],
                                                ps2[:, cb + 128:cb + w],
                                                0.125, None, MUL)
                    A2.append(a2)

                # O2^T (d-major)
                po2T = pot.tile([64, L], f32, tag="pot", name=f"po2T_{h}")
                for cs in range(NCH):
                    c0 = cs * 128
                    MM(po2T[:, c0:L], Et[:, cs, :], A2[cs][:, c0:L],
                                     start=(cs == 0), stop=(cs == NCH - 1))

                # combine: outT_h = gc_t * O1^T + O2^T
                m1 = sm.tile([64, L], f32, tag="m1", bufs=2, name=f"m1_{h}")
                nc.vector.tensor_tensor(m1[:], po1T[:], v_gcb[0:64, h, :], MUL)
                ot = att.tile([64, L], f32r, tag="ot", bufs=4, name=f"ot_{h}")
                nc.vector.tensor_tensor(ot[:], po2T[:], m1[:],
                                        mybir.AluOpType.add)
                outs.append(ot)

            # ---- output projection yT[n, t] = sum_h wcp[j(h), n] outT_h[j, t]
            # ACT mirror of head 0's output: the first matmul of every py
            # group then waits only on ACT (merged with the psA bank WAR).
            ot0a = qk.tile([64, L], f32r)
            nc.scalar.copy(ot0a[:], outs[0][:])
            rhs_h = [ot0a] + outs[1:]
            ysb = qk.tile([128, KC, L], f32)
            for nci in range(KC):
                n0 = nci * 128
                py = psA.tile([128, L], f32, tag="pbig", name=f"py_{nci}")
                for h in range(4):
                    mmp = MM(py[:], v_wcp[0:64, h, n0:n0 + 128],
                             rhs_h[h][:], start=(h == 0), stop=(h == 3))
                    if nci == 0 and h == 0:
                        add_dep_helper(mmp.ins, ldw_r3.ins, sync=False,
                                       reason="r3 anchor before out proj")
                nc.scalar.copy(ysb[:, nci, :], py[:])
            nc.sync.dma_start(
                out=yT.ap().rearrange("(c p) t -> p c t", p=128), in_=ysb[:])

    # this walrus build allows ONE sync wait per instruction; Tile's final
    # drain carries the whole vector clock, so split it into a chain of
    # single-wait drains (the SP sequencer executes them in order).
    if not split_waits:
        return nc
    nsplit = 0
    for bb in nc.m.functions[0].blocks:
        new_insts = []
        for inst in bb.instructions:
            si = getattr(inst, "sync_info", None)
            if si is not None and si.on_wait and len(si.on_wait) > 1:
                waits = list(si.on_wait)
                for j, w in enumerate(waits[:-1]):
                    d = mybir.InstDrain(
                        name=f"{inst.name}_sw{j}", engine=inst.engine,
                        ins=[], outs=[],
                        sync_info=mybir.SyncInfo(on_wait=[w], on_update=[]))
                    new_insts.append(d)
                    nsplit += 1
                si.on_wait = [waits[-1]]
            new_insts.append(inst)
        bb.instructions = new_insts
    return nc


# ----------------------------------------------------------------- host side
def _sigmoid(z):
    return 1.0 / (1.0 + np.exp(-z))


def _pack_core(core, x, Gc, U, wq_h, wk_h, wk2_h, wcp_h, bq_h, bk_h, bk2s_h,
               mi, ms8):
    b = core // 4
    h0 = (core % 4) * 4

    pk = np.zeros((128, TOT), dtype=np.float32)

    def put(name, arr):
        a, bb = _OFF[name]
        pk[:arr.shape[0], a:bb] = arr.reshape(arr.shape[0], -1)

    xb = x[b]                                        # (L, D)
    put("xT", xb.T.reshape(KC, 128, L).transpose(1, 0, 2))
    put("wq", wq_h[core])
    put("wk", wk_h[core])
    put("wk2", wk2_h[core])
    put("wcp", wcp_h[core])

    hsl = slice(h0 * DH, (h0 + 4) * DH)
    vh = xb[:, hsl].reshape(L, 4, DH)
    put("v", vh.reshape(NCH, 128, 4, DH).transpose(1, 2, 0, 3))
    vsh = np.zeros_like(vh)
    vsh[:-1] = vh[1:]
    put("vs", vsh.reshape(NCH, 128, 4, DH).transpose(1, 2, 0, 3))

    gch = Gc[b, :, h0:h0 + 4]                        # (L, 4)
    uh = U[b, :, h0:h0 + 4]
    put("gcb", np.broadcast_to(gch.T[None, :, :], (64, 4, L)))
    put("u", uh.reshape(NCH, 128, 4).transpose(1, 2, 0))
    put("gc", gch.reshape(NCH, 128, 4).transpose(1, 2, 0))
    put("bq", bq_h[core])
    put("bk", bk_h[core])
    put("bqr", 0.98 * bq_h[core])
    put("bk2s", bk2s_h[core])
    put("mi", mi)
    put("ms8", ms8)
    return pk


def kernel(**inputs):
    inputs = {k: np.asarray(v, dtype=np.float32) for k, v in inputs.items()}
    x = inputs["x"]
    q1_w, q1_b = inputs["q1_w"], inputs["q1_b"]
    k1_w, k1_b = inputs["k1_w"], inputs["k1_b"]
    k2_w, k2_b = inputs["k2_w"], inputs["k2_b"]
    gw_w, gw_b = inputs["gw_w"], inputs["gw_b"]
    sw_w, sw_b = inputs["sw_w"], inputs["sw_b"]
    cp_w, cp_b = inputs["cp_w"], inputs["cp_b"]

    # host scalars: gate cumprod Gc and u = silu(K@sw)/Gc  (tiny matvecs)
    xh = x.reshape(B, L, H, DH)
    zg = np.einsum("blhd,d->blh", xh, gw_w[:, 0], optimize=True) + gw_b[0]
    G = _sigmoid(zg)
    log_cp = np.clip(np.cumsum(np.log(np.clip(G, 1e-6, None)), axis=1),
                     -30.0, 30.0)
    Gc = (np.exp(log_cp) + 1e-6).astype(np.float32)        # (B, L, H)

    Wr = (k1_w.reshape(D, H, DH) @ sw_w[:, 0]).astype(np.float32)
    Cr = (k1_b.reshape(H, DH) @ sw_w[:, 0]) + sw_b[0]
    zr = x @ Wr + Cr
    R = zr * _sigmoid(zr)
    U = (R / Gc).astype(np.float32)

    idx = np.arange(128)
    mi = (idx[:, None] <= idx[None, :]).astype(np.float32)
    ms8 = 0.125 * (idx[:, None] < idx[None, :]).astype(np.float32)
    c2 = np.float32(0.02 / np.sqrt(np.float32(D)))

    wq_h, wk_h, wk2_h, wcp_h, bq_h, bk_h, bk2s_h = {}, {}, {}, {}, {}, {}, {}
    for core in range(N_CORES):
        h0 = (core % 4) * 4
        hsl = slice(h0 * DH, (h0 + 4) * DH)
        wq_h[core] = q1_w[:, hsl].reshape(KC, 128, 256).transpose(1, 0, 2)
        wk_h[core] = k1_w[:, hsl].reshape(KC, 128, 256).transpose(1, 0, 2)
        wk2_h[core] = k2_w[:, hsl].reshape(KC, 128, 256).transpose(1, 0, 2)
        wcp_h[core] = cp_w[hsl, :].reshape(4, 64, D).transpose(1, 0, 2)
        bq_h[core] = q1_b[hsl].reshape(2, 128).T
        bk_h[core] = k1_b[hsl].reshape(2, 128).T
        bk2s_h[core] = (k2_b[hsl] * c2).reshape(2, 128).T

    in_maps = [
        {"inp": _pack_core(core, x, Gc, U, wq_h, wk_h, wk2_h, wcp_h,
                           bq_h, bk_h, bk2s_h, mi, ms8)}
        for core in range(N_CORES)
    ]

    res = _run(in_maps)

    y = np.empty((B, L, D), dtype=np.float32)
    for b in range(B):
        acc = res[4 * b]["yT"].astype(np.float32)
        for c in range(4 * b + 1, 4 * b + 4):
            acc = acc + res[c]["yT"]
        y[b] = acc.T
    y += cp_b
    return y


def _run(in_maps, trace=False):
    if "nc" not in _CACHE:
        _CACHE["nc"] = _build_nc()
    from concourse.bass_utils import run_bass_kernel_spmd
    r = run_bass_kernel_spmd(_CACHE["nc"], in_maps,
                             core_ids=list(range(N_CORES)), trace=trace)
    _CACHE["last"] = r
    return r.results
